# revision 34
# baseline (speedup 1.0000x reference)
"""Trainium2 Bass kernel for causal GQA self-attention with QK RMS-norm.

Problem (hardcoded): B=2, T=2048, d_model=2048, 16 Q heads / 4 KV heads,
head_dim=128, fp32 IO.

Sharding across 8 NeuronCores: tensor-parallel over the 4 KV head groups
(each group = 1 KV head + its 4 Q heads) x data-parallel over the 2
batches.  core = 4*b + g.  Each core computes
    qkvT_g = Wg.T @ x_b.T          ([768, T])
    q/k RMS-norm (+ per-dim scales), v transposed to natural layout
    causal attention for the 4 Q heads of group g (S^T orientation)
    yp_partial = (O^T).T @ Wp_g    ([T, d_model] partial)
and the host sums the 4 partials per batch.
"""

import functools

import numpy as np

import concourse.bass as bass
import concourse.mybir as mybir
import concourse.tile as tile
from concourse import bacc
from concourse.bass_utils import run_bass_kernel_spmd
from concourse.masks import make_identity

F32 = mybir.dt.float32
BF16 = mybir.dt.bfloat16
F32R = mybir.dt.float32r

# matmul operand dtype (stationary+moving). BF16 halves weight-load time
# (and enables FWL); fp32r is the higher-precision fallback.
MM_DT = BF16

T = 2048
C = 2048
D = 128
NH = 4            # q heads per core
NKC = C // 128    # 16 k-chunks of the d_model contraction
QKV = (NH + 2) * D  # 768 qkv rows per core
NT = 4            # 512-wide q/T tiles
TQ = 512
NEG = -1.0e30
EPS = 1e-6
SM_SCALE = 1.0 / float(np.sqrt(D))


def _pack_batches(q0, qw):
    """S^T j-chunk batches for the query window [q0, q0+qw).

    Each batch is a list of (j, co, width, pos): j = key chunk, co = start
    column inside the window, width = qw - co, pos = column of the block
    inside the packed PSUM batch tile.  Blocks never cross a 512-column
    PSUM bank boundary.
    """
    chunks = []
    j = 0
    while 128 * j < q0 + qw:
        co = max(0, 128 * j - q0)
        chunks.append((j, co, qw - co))
        j += 1
    batches = []
    cur, pos = [], 0
    for (j, co, w) in chunks:
        if pos // 512 != (pos + w - 1) // 512 and pos % 512 != 0:
            pos = (pos // 512 + 1) * 512
        if pos + w > 1024:
            batches.append(cur)
            cur, pos = [], 0
        cur.append((j, co, w, pos))
        pos += w
    if cur:
        batches.append(cur)
    return batches


# attention/proj query windows: three 512-wide, then two 256-wide so the
# non-overlappable final proj (all heads of the last window must finish
# before it) is half as long
WINDOWS = [(0, 512), (512, 512), (1024, 512), (1536, 256), (1792, 256)]


def build_kernel():
    nc = bacc.Bacc()
    xT_d = nc.dram_tensor("xT", [C, T], MM_DT, kind="ExternalInput")
    # wg comes host-packed as [p, kc, q] so multi-chunk DMAs read long
    # contiguous lines (2 chunks = 3KB/partition vs 1.5KB), keeping DMA
    # ahead of the warmed-up tensor engine during tile 0
    wg_d = nc.dram_tensor("wg", [128, NKC * QKV], MM_DT, kind="ExternalInput")
    wp_d = nc.dram_tensor("wp", [NH * D, C], MM_DT, kind="ExternalInput")
    qks_d = nc.dram_tensor("qks", [D, 2], F32, kind="ExternalInput")
    # bf16 output halves the store traffic; the host sums the four
    # per-group partials in fp32 (0.4% rounding on partials is well
    # inside the 2e-2 tolerance)
    out_d = nc.dram_tensor("out", [T, C], BF16, kind="ExternalOutput")

    xT_r = xT_d.rearrange("(kc p) t -> p kc t", p=128)
    wg_r = wg_d.rearrange("p (kc q) -> p kc q", kc=NKC)

    with tile.TileContext(nc) as tc:
        with (
            tc.tile_pool(name="consts", bufs=1) as consts,
            tc.tile_pool(name="qkv_sb", bufs=1) as qkv_sb,
        ):
            # ---- constants (emitted after the kc=0 DMA triggers below) ----
            ident = consts.tile([128, 128], MM_DT)
            ones32 = consts.tile([128, 128], F32)
            ones_m = consts.tile([128, 128], MM_DT)
            eps_t = consts.tile([128, 1], F32)
            qks_t = consts.tile([128, 2], F32)

            # ---- persistent activations, split per 512-wide tile so the
            # attention on tile n only depends on stage A tile n ----
            qTn = [
                qkv_sb.tile([128, NH, TQ], MM_DT, name=f"qT{n}")
                for n in range(NT)
            ]
            kTn = [
                qkv_sb.tile([128, TQ], MM_DT, name=f"kT{n}")
                for n in range(NT)
            ]
            vn = [
                qkv_sb.tile([128, 4, 128], MM_DT, name=f"v{n}")
                for n in range(NT)
            ]
            oTn = [
                qkv_sb.tile([128, NH, TQ], MM_DT, name=f"oT{n}")
                for n in range(NT)
            ]
            wp_sb = qkv_sb.tile([128, NH, C], MM_DT)
            wp_r = wp_d.rearrange("(h p) c -> p h c", p=128)

            # ================= Stage A: qkvT = Wg.T @ xT =================
            with (
                tc.tile_pool(name="wg_pool", bufs=1) as wg_pool,
                tc.tile_pool(name="xt_pool", bufs=2) as xt_pool,
                tc.tile_pool(name="normtmp", bufs=4) as normtmp,
                tc.tile_pool(name="vtmp", bufs=2) as vtmp,
            ):
                # per-chunk loads so the first matmuls start early; triggers
                # rotate across queue engines (a single engine issues a
                # DMA trigger only every ~650ns, which serializes startup)
                wg_sb = wg_pool.tile([128, NKC, QKV], MM_DT)
                xts = []
                for n in range(NT):
                    xts.append(
                        xt_pool.tile(
                            [128, NKC, TQ], MM_DT, tag="xt", name=f"xt{n}"
                        )
                    )
                # kc=0 first and split across three prompt trigger engines
                # so the first matmul's operands land ASAP
                # first matmul needs xt kc0 (both halves) + wg kc0[0:128];
                # split across all three trigger engines so the critical
                # pieces transfer in parallel
                nc.sync.dma_start(
                    out=xts[0][:, 0, 0:256], in_=xT_r[:, 0, 0:256]
                )
                nc.scalar.dma_start(
                    out=xts[0][:, 0, 256:TQ], in_=xT_r[:, 0, 256:TQ]
                )
                nc.gpsimd.dma_start(
                    out=wg_sb[:, 0, 0:384], in_=wg_r[:, 0, 0:384]
                )
                nc.gpsimd.dma_start(
                    out=wg_sb[:, 0, 384:768], in_=wg_r[:, 0, 384:768]
                )
                # gpsimd: identity + ~1.2us dummy delay its kc triggers so
                # the kc=0 pieces transfer without competing traffic
                make_identity(nc, ident)
                junk = normtmp.tile([128, TQ], F32, tag="junk")
                nc.gpsimd.memset(junk, 0.0)
                # vector consts after its single kc0 trigger
                nc.vector.memset(ones32, 1.0)
                nc.vector.tensor_copy(ones_m, ones32)
                nc.vector.memset(eps_t, EPS)
                # ACT table-set preloads: first use of each activation set
                # costs ~2.7us (table DMA + drain); pay it here during the
                # DMA wait instead of mid-kernel
                # ACT table RAM holds only TWO function sets; preload the
                # stage-A pair (Square/Sqrt) here.  Exp/Copy are warmed at
                # the end of stage A, after the last Sqrt, so no load ever
                # lands on the critical first-exp path.
                tblw = normtmp.tile([128, 1], F32, tag="tblw")
                for fn in (
                    mybir.ActivationFunctionType.Square,
                    mybir.ActivationFunctionType.Sqrt,
                ):
                    nc.scalar.activation(out=tblw, in_=eps_t, func=fn)
                # wg streams in 2-chunk pieces (3KB contiguous lines from
                # the packed layout); xt per-chunk; round-robin triggers
                trig = [nc.sync, nc.scalar, nc.gpsimd]
                # 2-chunk pieces: 3KB contiguous lines from the packed
                # layout keep aggregate DMA bandwidth high
                wg_pieces = [(1, 3), (3, 5), (5, 7), (7, 9), (9, 11),
                             (11, 13), (13, 15), (15, 16)]
                ti = 0
                for pi, (ka, kb) in enumerate(wg_pieces):
                    trig[ti % 3].dma_start(
                        out=wg_sb[:, ka:kb, :], in_=wg_r[:, ka:kb, :]
                    )
                    ti += 1
                    for kc in range(ka, kb):
                        trig[ti % 3].dma_start(
                            out=xts[0][:, kc, :], in_=xT_r[:, kc, 0:TQ]
                        )
                        ti += 1
                # q/k scales are first needed by the norm chain ~15us in
                nc.sync.dma_start(out=qks_t, in_=qks_d[:, :])

                with (
                    tc.tile_pool(name="psA", bufs=1, space="PSUM") as psA,
                    tc.tile_pool(name="psN", bufs=1, space="PSUM") as psN,
                    tc.tile_pool(name="psV", bufs=1, space="PSUM") as psV,
                ):
                    def process_m(n, m, acc):
                        if m < 5:
                            # rms over partition dim via ones-matmul bcast
                            sq = normtmp.tile([128, TQ], MM_DT, tag="sq")
                            nc.scalar.activation(
                                out=sq, in_=acc,
                                func=mybir.ActivationFunctionType.Square,
                            )
                            # stage acc to SBUF (DVE) so its PSUM slot
                            # frees after two fast ops instead of after
                            # the whole normalize chain
                            araw = normtmp.tile([128, TQ], F32, tag="araw")
                            nc.vector.tensor_copy(araw, acc)
                            ssq = psN.tile([128, TQ], F32, tag="ssq")
                            nc.tensor.matmul(ssq, lhsT=ones_m, rhs=sq)
                            rms = normtmp.tile([128, TQ], F32, tag="rms")
                            nc.scalar.activation(
                                out=rms, in_=ssq,
                                func=mybir.ActivationFunctionType.Sqrt,
                                bias=eps_t, scale=1.0 / D,
                            )
                            rinv = normtmp.tile([128, TQ], F32, tag="rinv")
                            nc.vector.reciprocal_approx_fast(out=rinv, in_=rms)
                            rsc = normtmp.tile([128, TQ], F32, tag="rsc")
                            nc.vector.tensor_scalar_mul(
                                out=rsc, in0=rinv,
                                scalar1=qks_t[:, 0:1] if m < 4
                                else qks_t[:, 1:2],
                            )
                            dst = qTn[n][:, m, :] if m < 4 else kTn[n][:, :]
                            nc.vector.tensor_mul(dst, araw, rsc)
                        else:
                            # v block: transpose to natural [tk, d].  The
                            # staging copy runs on ACT: at the stage-A tail
                            # the DVE still has the k-norm chain queued,
                            # and the PE's transposes would stall behind a
                            # DVE copy (ACT is idle there)
                            vt = vtmp.tile([128, TQ], MM_DT, tag="vt")
                            nc.scalar.copy(vt, acc)
                            for jj in range(4):
                                vps = psV.tile([128, 128], MM_DT, tag="vps")
                                nc.tensor.transpose(
                                    vps, vt[:, jj * 128:(jj + 1) * 128], ident
                                )
                                nc.vector.tensor_copy(vn[n][:, jj, :], vps)

                    for n in range(NT):
                        xt_sb = xts[n]
                        if n + 1 < NT:
                            for g4 in range(4):
                                nc.sync.dma_start(
                                    out=xts[n + 1][:, 4 * g4:4 * g4 + 4, :],
                                    in_=xT_r[
                                        :, 4 * g4:4 * g4 + 4,
                                        (n + 1) * TQ:(n + 2) * TQ
                                    ],
                                )
                        if n == 1:
                            # wp prefetch: late enough not to delay xt tiles,
                            # early enough to be resident before proj starts
                            for h in range(NH):
                                nc.sync.dma_start(
                                    out=wp_sb[:, h, :], in_=wp_r[:, h, :]
                                )
                        if n == 0:
                            # tile 0 is DMA-paced: stream chunks into 6
                            # parallel accumulators so consumption matches
                            # per-chunk arrival order
                            accs = [
                                psA.tile(
                                    [128, TQ], F32,
                                    tag=f"acc{m}", name=f"acc{m}",
                                )
                                for m in range(6)
                            ]
                            for kc in range(NKC):
                                for m in range(6):
                                    nc.tensor.matmul(
                                        accs[m],
                                        lhsT=wg_sb[
                                            :, kc, m * 128:(m + 1) * 128
                                        ],
                                        rhs=xt_sb[:, kc, :],
                                        start=(kc == 0),
                                        stop=(kc == NKC - 1),
                                    )
                                if kc <= 5:
                                    # tile 0 is DMA-paced with ~zero slack
                                    # and the first chunks arrive slowest;
                                    # throwaway matmuls into the idle ssq
                                    # bank give the DMA stream a lead so
                                    # the PE never starves (a starve-gap
                                    # re-throttles the clock-gate)
                                    warm = psN.tile(
                                        [128, TQ], F32, tag="ssq"
                                    )
                                    for _ in range(2 if kc <= 1 else 1):
                                        nc.tensor.matmul(
                                            warm, lhsT=ident,
                                            rhs=xt_sb[:, kc, :],
                                            start=True, stop=True,
                                        )
                            for m in range(6):
                                process_m(n, m, accs[m])
                        else:
                            # tiles 1-3: m's in groups, so each group's
                            # norm chains drain (and free their PSUM banks)
                            # while the next group accumulates.  This also
                            # frees the attention-phase banks (reused from
                            # psA in order) progressively, killing the
                            # stage A -> attention stall that re-throttled
                            # the PE clock-gate.  The last tile splits
                            # (4,5) into two solo groups: m4's ACT chain
                            # and the Exp/Copy table warms drain during
                            # m5's accumulation, so ACT is free for the
                            # first exps when attention starts.
                            groups = (
                                [(0, 1), (2, 3), (4,), (5,)]
                                if n == NT - 1
                                else [(0, 1), (2, 3), (4, 5)]
                            )
                            for ms in groups:
                                accs = {
                                    m: psA.tile(
                                        [128, TQ], F32,
                                        tag=f"acc{m}", name=f"acc{m}",
                                    )
                                    for m in ms
                                }
                                for kc in range(NKC):
                                    for m in ms:
                                        nc.tensor.matmul(
                                            accs[m],
                                            lhsT=wg_sb[
                                                :, kc, m * 128:(m + 1) * 128
                                            ],
                                            rhs=xt_sb[:, kc, :],
                                            start=(kc == 0),
                                            stop=(kc == NKC - 1),
                                        )
                                for m in ms:
                                    process_m(n, m, accs[m])
                                if n == NT - 1 and ms == (4,):
                                    # warm the Exp and Copy table sets now
                                    # (evicting Square/Sqrt, which are done
                                    # for good); overlaps m5 accumulation
                                    for fn in (
                                        mybir.ActivationFunctionType.Exp,
                                        mybir.ActivationFunctionType.Copy,
                                    ):
                                        nc.scalar.activation(
                                            out=tblw, in_=eps_t, func=fn
                                        )

            # ============ Attention + proj, per 512-wide q tile ============
            with (
                tc.tile_pool(name="pt_pool", bufs=4) as pt_pool,
                tc.tile_pool(name="pa_pool", bufs=3) as pa_pool,
                tc.tile_pool(name="rs_pool", bufs=3) as rs_pool,
                tc.tile_pool(name="yp_pool", bufs=2) as yp_pool,
                tc.tile_pool(name="psS", bufs=2, space="PSUM") as psS,
                tc.tile_pool(name="psO", bufs=2, space="PSUM") as psO,
                tc.tile_pool(name="psC", bufs=2, space="PSUM") as psC,
            ):
                def emit_proj(q0, qw):
                    # y[window] = (oT).T @ wp for this query window
                    for tt in range(qw // 128):
                        c0 = q0 + tt * 128
                        n, no = c0 // TQ, c0 % TQ
                        # the final row tile streams out per-512-col piece
                        # so the last output DMA is a quarter of the size
                        split_dma = (c0 == T - 128)
                        yp = yp_pool.tile([128, C], BF16, tag="yp", name="yp")
                        for cn in range(4):
                            pc = psC.tile([128, TQ], F32, tag="pc", name="pc")
                            for h in range(NH):
                                nc.tensor.matmul(
                                    pc,
                                    lhsT=oTn[n][:, h, no:no + 128],
                                    rhs=wp_sb[:, h, cn * TQ:(cn + 1) * TQ],
                                    start=(h == 0),
                                    stop=(h == NH - 1),
                                )
                            # alternate copy engines so neither ACT nor DVE
                            # becomes the drain bottleneck; window (1536,)
                            # drains on ACT only (the DVE still has its
                            # p_acc/normalize chains), but the final window
                            # alternates again — at the very end both
                            # engines drain in parallel
                            if q0 == 1536 or cn % 2 == 0:
                                nc.scalar.copy(yp[:, cn * TQ:(cn + 1) * TQ], pc)
                            else:
                                nc.vector.tensor_copy(
                                    yp[:, cn * TQ:(cn + 1) * TQ], pc
                                )
                            if split_dma:
                                nc.sync.dma_start(
                                    out=out_d[
                                        c0:c0 + 128, cn * TQ:(cn + 1) * TQ
                                    ],
                                    in_=yp[:, cn * TQ:(cn + 1) * TQ],
                                )
                        if not split_dma:
                            nc.sync.dma_start(out=out_d[c0:c0 + 128, :], in_=yp)

                fin_q = []

                def finalize(item):
                    # one ones-matmul over the accumulated p_acc gives
                    # the softmax denominator (replaces the per-chunk row-sum
                    # matmuls), then normalize; emit proj for the window
                    # after its last head.
                    o_ps, p_acc, q0, qw, h = item
                    n, qo = q0 // TQ, q0 % TQ
                    # u shares the psC ring (saves a PSUM bank for psO=2;
                    # psS's ring parity must stay untouched)
                    u_ps = psC.tile([128, TQ], F32, tag="pc", name="u_ps")
                    nc.tensor.matmul(
                        u_ps[:, 0:qw], lhsT=ones_m, rhs=p_acc[:, 0:qw]
                    )
                    rsum = rs_pool.tile(
                        [128, TQ], F32, tag="rsum", name="rsum"
                    )
                    nc.vector.reciprocal_approx_fast(
                        out=rsum[:, 0:qw], in_=u_ps[:, 0:qw]
                    )
                    nc.vector.tensor_mul(
                        oTn[n][:, h, qo:qo + qw], o_ps[:, 0:qw], rsum[:, 0:qw]
                    )
                    if h == NH - 1:
                        emit_proj(q0, qw)

                def flush(item):
                    # PV matmuls for a finished exp batch. The head's
                    # finalize (denominator + normalize + proj) is deferred
                    # two further flushes so the exp->mask->p_acc-add chain
                    # of the diagonal batch is done when the u matmul
                    # issues — but no longer, so each window's proj burst
                    # starts early enough to feed the PE through the next
                    # window's exp-throughput-bound opening heads.
                    batch, p_sb, o_ps, p_acc, q0, qw, h, is_last, jmax = item
                    for ent in fin_q:
                        ent[1] += 1
                    while fin_q and fin_q[0][1] >= 2:
                        finalize(fin_q.pop(0)[0])
                    for (j, co, w, pos) in batch:
                        nc.tensor.matmul(
                            o_ps[:, co:co + w],
                            lhsT=vn[j // 4][:, j % 4, :],
                            rhs=p_sb[:, pos:pos + w],
                            start=(j == 0),
                            stop=(j == jmax),
                        )
                    # p_acc accumulation emitted here (2 batches after the
                    # exp/masks) so the DVE queue never sits waiting on a
                    # just-issued exp or gpsimd mask — an inline wait stalls
                    # every DVE op queued behind it.  (Keep this on DVE:
                    # gpsimd tensor ops measured 2-6x slower and its FIFO
                    # would delay the causal masks that gate PV.)
                    for (j, co, w, pos) in batch:
                        if j == 0:
                            nc.vector.tensor_copy(
                                p_acc[:, 0:qw], p_sb[:, pos:pos + w]
                            )
                        else:
                            nc.vector.tensor_add(
                                p_acc[:, co:co + w],
                                p_acc[:, co:co + w],
                                p_sb[:, pos:pos + w],
                            )
                    if is_last:
                        fin_q.append([(o_ps, p_acc, q0, qw, h), 0])

                # depth-2 software pipeline: flush batch i-2 while batch i's
                # scores stream and batch i-1's exp runs on ACT, so the PV
                # matmuls never wait on a just-issued exp
                import collections
                pending = collections.deque()
                first_flush = [True]

                def fill_bridge(n_mm):
                    # throwaway matmuls into ONE psC slot (the first slot
                    # reuses psN's old bank, free well before the boundary;
                    # the second slot was psV's, which the v-transposes
                    # hold until the very end of stage A): keep the PE (and
                    # its HAM clock-gate) busy while the S->exp->PV
                    # pipeline fills or the final p_acc chains drain
                    dum = psC.tile([128, TQ], F32, tag="pc", name="pc")
                    for i in range(n_mm):
                        nc.tensor.matmul(
                            dum, lhsT=ones_m, rhs=kTn[0][:, :],
                            start=True, stop=True,
                        )

                for (q0, qw) in WINDOWS:
                    batches = _pack_batches(q0, qw)
                    jmax = (q0 + qw) // 128 - 1
                    n, qo = q0 // TQ, q0 % TQ
                    for h in range(NH):
                        o_ps = psO.tile([128, TQ], F32, tag="o", name="o_ps")
                        p_acc = pa_pool.tile(
                            [128, TQ], MM_DT, tag="pa", name="p_acc"
                        )
                        for bi, batch in enumerate(batches):
                            bw = batch[-1][3] + batch[-1][2]
                            s_ps = psS.tile(
                                [128, 1024], F32, tag="s", name="s_ps"
                            )
                            for (j, co, w, pos) in batch:
                                nc.tensor.matmul(
                                    s_ps[:, pos:pos + w],
                                    lhsT=kTn[j // 4][
                                        :, (j % 4) * 128:(j % 4 + 1) * 128
                                    ],
                                    rhs=qTn[n][:, h, qo + co:qo + qw],
                                )
                            p_sb = pt_pool.tile(
                                [128, 1024], MM_DT, tag="p", name="p_sb"
                            )
                            nc.scalar.activation(
                                out=p_sb[:, 0:bw], in_=s_ps[:, 0:bw],
                                func=mybir.ActivationFunctionType.Exp,
                                scale=SM_SCALE,
                            )
                            for (j, co, w, pos) in batch:
                                if 128 * j >= q0:
                                    # causal mask: zero p where col < row
                                    # (gpsimd: otherwise-idle engine)
                                    nc.gpsimd.affine_select(
                                        out=p_sb[:, pos:pos + 128],
                                        in_=p_sb[:, pos:pos + 128],
                                        pattern=[[1, 128]],
                                        channel_multiplier=-1, base=0,
                                        compare_op=mybir.AluOpType.is_ge,
                                        fill=0.0,
                                    )
                            pending.append((
                                batch, p_sb, o_ps, p_acc, q0, qw, h,
                                bi == len(batches) - 1, jmax,
                            ))
                            if len(pending) > 2:
                                if first_flush[0]:
                                    # bridge the attention pipeline fill:
                                    # the first PV waits S0+exp0 (~1.6us)
                                    first_flush[0] = False
                                    fill_bridge(4)
                                flush(pending.popleft())
                while pending:
                    flush(pending.popleft())
                # bridge the drain: the last u matmuls wait the final
                # p_acc chains on the DVE (~2.5us)
                fill_bridge(12)
                while fin_q:
                    finalize(fin_q.pop(0)[0])

    nc.finalize()
    return nc


@functools.lru_cache(maxsize=1)
def _get_nc():
    return build_kernel()


def make_in_maps(x, W_qkv, W_proj, q_scale, k_scale):
    x = np.asarray(x, dtype=np.float32)
    W_qkv = np.asarray(W_qkv, dtype=np.float32)
    W_proj = np.asarray(W_proj, dtype=np.float32)
    q_scale = np.asarray(q_scale, dtype=np.float32)
    k_scale = np.asarray(k_scale, dtype=np.float32)

    import ml_dtypes

    bf16 = ml_dtypes.bfloat16
    qks = np.ascontiguousarray(
        np.stack([q_scale, k_scale], axis=1).astype(np.float32)
    )
    xT_by_batch = [np.ascontiguousarray(x[b].T).astype(bf16) for b in range(2)]
    in_maps = []
    for core in range(8):
        b, g = divmod(core, 4)
        wg_cols = np.concatenate(
            [
                W_qkv[:, 512 * g:512 * (g + 1)],
                W_qkv[:, 2048 + 128 * g:2048 + 128 * (g + 1)],
                W_qkv[:, 2560 + 128 * g:2560 + 128 * (g + 1)],
            ],
            axis=1,
        )
        # pack to [p, kc, q] so per-partition DRAM lines span kc chunks
        wg = np.ascontiguousarray(
            wg_cols.reshape(16, 128, 768).transpose(1, 0, 2).reshape(128, -1)
        ).astype(bf16)
        wp = np.ascontiguousarray(W_proj[512 * g:512 * (g + 1), :]).astype(bf16)
        in_maps.append(
            {"xT": xT_by_batch[b], "wg": wg, "wp": wp, "qks": qks}
        )
    return in_maps


def kernel(x, W_qkv, W_proj, q_scale, k_scale):
    nc = _get_nc()
    in_maps = make_in_maps(x, W_qkv, W_proj, q_scale, k_scale)
    res = run_bass_kernel_spmd(nc, in_maps, core_ids=list(range(8)))
    outs = [np.asarray(r["out"], dtype=np.float32) for r in res.results]
    y0 = outs[0] + outs[1] + outs[2] + outs[3]
    y1 = outs[4] + outs[5] + outs[6] + outs[7]
    return np.stack([y0, y1], axis=0).astype(np.float32)


# revision 35
# speedup vs baseline: 1.0244x; 1.0244x over previous
"""Trainium2 Bass kernel for causal GQA self-attention with QK RMS-norm.

Problem (hardcoded): B=2, T=2048, d_model=2048, 16 Q heads / 4 KV heads,
head_dim=128, fp32 IO.

Sharding across 8 NeuronCores: tensor-parallel over the 4 KV head groups
(each group = 1 KV head + its 4 Q heads) x data-parallel over the 2
batches.  core = 4*b + g.  Each core computes
    qkvT_g = Wg.T @ x_b.T          ([768, T])
    q/k RMS-norm (+ per-dim scales), v transposed to natural layout
    causal attention for the 4 Q heads of group g (S^T orientation)
    yp_partial = (O^T).T @ Wp_g    ([T, d_model] partial)
and the host sums the 4 partials per batch.
"""

import functools

import numpy as np

import concourse.bass as bass
import concourse.mybir as mybir
import concourse.tile as tile
from concourse import bacc
from concourse.bass_utils import run_bass_kernel_spmd
from concourse.masks import make_identity

F32 = mybir.dt.float32
BF16 = mybir.dt.bfloat16
F32R = mybir.dt.float32r

# matmul operand dtype (stationary+moving). BF16 halves weight-load time
# (and enables FWL); fp32r is the higher-precision fallback.
MM_DT = BF16

T = 2048
C = 2048
D = 128
NH = 4            # q heads per core
NKC = C // 128    # 16 k-chunks of the d_model contraction
QKV = (NH + 2) * D  # 768 qkv rows per core
NT = 4            # 512-wide q/T tiles
TQ = 512
NEG = -1.0e30
EPS = 1e-6
SM_SCALE = 1.0 / float(np.sqrt(D))


def _pack_batches(q0, qw):
    """S^T j-chunk batches for the query window [q0, q0+qw).

    Each batch is a list of (j, co, width, pos): j = key chunk, co = start
    column inside the window, width = qw - co, pos = column of the block
    inside the packed PSUM batch tile.  Blocks never cross a 512-column
    PSUM bank boundary.
    """
    chunks = []
    j = 0
    while 128 * j < q0 + qw:
        co = max(0, 128 * j - q0)
        chunks.append((j, co, qw - co))
        j += 1
    batches = []
    cur, pos = [], 0
    for (j, co, w) in chunks:
        if pos // 512 != (pos + w - 1) // 512 and pos % 512 != 0:
            pos = (pos // 512 + 1) * 512
        if pos + w > 1024:
            batches.append(cur)
            cur, pos = [], 0
        cur.append((j, co, w, pos))
        pos += w
    if cur:
        batches.append(cur)
    return batches


# attention/proj query windows: three 512-wide, then two 256-wide so the
# non-overlappable final proj (all heads of the last window must finish
# before it) is half as long
WINDOWS = [(0, 512), (512, 512), (1024, 512), (1536, 256), (1792, 256)]


def build_kernel():
    nc = bacc.Bacc()
    xT_d = nc.dram_tensor("xT", [C, T], MM_DT, kind="ExternalInput")
    # wg comes host-packed as [p, kc, q] so multi-chunk DMAs read long
    # contiguous lines (2 chunks = 3KB/partition vs 1.5KB), keeping DMA
    # ahead of the warmed-up tensor engine during tile 0
    wg_d = nc.dram_tensor("wg", [128, NKC * QKV], MM_DT, kind="ExternalInput")
    wp_d = nc.dram_tensor("wp", [NH * D, C], MM_DT, kind="ExternalInput")
    qks_d = nc.dram_tensor("qks", [D, 2], F32, kind="ExternalInput")
    # bf16 output halves the store traffic; the host sums the four
    # per-group partials in fp32 (0.4% rounding on partials is well
    # inside the 2e-2 tolerance)
    out_d = nc.dram_tensor("out", [T, C], BF16, kind="ExternalOutput")

    xT_r = xT_d.rearrange("(kc p) t -> p kc t", p=128)
    wg_r = wg_d.rearrange("p (kc q) -> p kc q", kc=NKC)

    with tile.TileContext(nc) as tc:
        with (
            tc.tile_pool(name="consts", bufs=1) as consts,
            tc.tile_pool(name="qkv_sb", bufs=1) as qkv_sb,
        ):
            # ---- constants (emitted after the kc=0 DMA triggers below) ----
            ident = consts.tile([128, 128], MM_DT)
            ones32 = consts.tile([128, 128], F32)
            ones_m = consts.tile([128, 128], MM_DT)
            eps_t = consts.tile([128, 1], F32)
            qks_t = consts.tile([128, 2], F32)

            # ---- persistent activations, split per 512-wide tile so the
            # attention on tile n only depends on stage A tile n ----
            qTn = [
                qkv_sb.tile([128, NH, TQ], MM_DT, name=f"qT{n}")
                for n in range(NT)
            ]
            kTn = [
                qkv_sb.tile([128, TQ], MM_DT, name=f"kT{n}")
                for n in range(NT)
            ]
            vn = [
                qkv_sb.tile([128, 4, 128], MM_DT, name=f"v{n}")
                for n in range(NT)
            ]
            oTn = [
                qkv_sb.tile([128, NH, TQ], MM_DT, name=f"oT{n}")
                for n in range(NT)
            ]
            wp_sb = qkv_sb.tile([128, NH, C], MM_DT)
            wp_r = wp_d.rearrange("(h p) c -> p h c", p=128)

            # ================= Stage A: qkvT = Wg.T @ xT =================
            with (
                tc.tile_pool(name="wg_pool", bufs=1) as wg_pool,
                tc.tile_pool(name="xt_pool", bufs=2) as xt_pool,
                tc.tile_pool(name="normtmp", bufs=4) as normtmp,
                tc.tile_pool(name="vtmp", bufs=2) as vtmp,
            ):
                # per-chunk loads so the first matmuls start early; triggers
                # rotate across queue engines (a single engine issues a
                # DMA trigger only every ~650ns, which serializes startup)
                wg_sb = wg_pool.tile([128, NKC, QKV], MM_DT)
                xts = []
                for n in range(NT):
                    xts.append(
                        xt_pool.tile(
                            [128, NKC, TQ], MM_DT, tag="xt", name=f"xt{n}"
                        )
                    )
                # kc=0 first and split across three prompt trigger engines
                # so the first matmul's operands land ASAP
                nc.sync.dma_start(out=xts[0][:, 0, :], in_=xT_r[:, 0, 0:TQ])
                nc.scalar.dma_start(
                    out=wg_sb[:, 0, 0:384], in_=wg_r[:, 0, 0:384]
                )
                nc.gpsimd.dma_start(
                    out=wg_sb[:, 0, 384:768], in_=wg_r[:, 0, 384:768]
                )
                # gpsimd: identity + ~1.2us dummy delay its kc triggers so
                # the kc=0 pieces transfer without competing traffic
                make_identity(nc, ident)
                junk = normtmp.tile([128, TQ], F32, tag="junk")
                nc.gpsimd.memset(junk, 0.0)
                # vector consts after its single kc0 trigger
                nc.vector.memset(ones32, 1.0)
                nc.vector.tensor_copy(ones_m, ones32)
                nc.vector.memset(eps_t, EPS)
                # ACT table-set preloads: first use of each activation set
                # costs ~2.7us (table DMA + drain); pay it here during the
                # DMA wait instead of mid-kernel
                # ACT table RAM holds only TWO function sets; preload the
                # stage-A pair (Square/Sqrt) here.  Exp/Copy are warmed at
                # the end of stage A, after the last Sqrt, so no load ever
                # lands on the critical first-exp path.
                tblw = normtmp.tile([128, 1], F32, tag="tblw")
                for fn in (
                    mybir.ActivationFunctionType.Square,
                    mybir.ActivationFunctionType.Sqrt,
                ):
                    nc.scalar.activation(out=tblw, in_=eps_t, func=fn)
                # wg streams in 2-chunk pieces (3KB contiguous lines from
                # the packed layout); xt per-chunk; round-robin triggers
                trig = [nc.sync, nc.scalar, nc.gpsimd]
                # 2-chunk pieces: 3KB contiguous lines from the packed
                # layout keep aggregate DMA bandwidth high
                wg_pieces = [(1, 3), (3, 5), (5, 7), (7, 9), (9, 11),
                             (11, 13), (13, 15), (15, 16)]
                ti = 0
                for pi, (ka, kb) in enumerate(wg_pieces):
                    trig[ti % 3].dma_start(
                        out=wg_sb[:, ka:kb, :], in_=wg_r[:, ka:kb, :]
                    )
                    ti += 1
                    for kc in range(ka, kb):
                        trig[ti % 3].dma_start(
                            out=xts[0][:, kc, :], in_=xT_r[:, kc, 0:TQ]
                        )
                        ti += 1
                # q/k scales are first needed by the norm chain ~15us in
                nc.sync.dma_start(out=qks_t, in_=qks_d[:, :])

                with (
                    tc.tile_pool(name="psA", bufs=1, space="PSUM") as psA,
                    tc.tile_pool(name="psN", bufs=1, space="PSUM") as psN,
                    tc.tile_pool(name="psV", bufs=1, space="PSUM") as psV,
                ):
                    def process_m(n, m, acc):
                        if m < 5:
                            # rms over partition dim via ones-matmul bcast
                            sq = normtmp.tile([128, TQ], MM_DT, tag="sq")
                            nc.scalar.activation(
                                out=sq, in_=acc,
                                func=mybir.ActivationFunctionType.Square,
                            )
                            # stage acc to SBUF (DVE) so its PSUM slot
                            # frees after two fast ops instead of after
                            # the whole normalize chain
                            araw = normtmp.tile([128, TQ], F32, tag="araw")
                            nc.vector.tensor_copy(araw, acc)
                            ssq = psN.tile([128, TQ], F32, tag="ssq")
                            nc.tensor.matmul(ssq, lhsT=ones_m, rhs=sq)
                            rms = normtmp.tile([128, TQ], F32, tag="rms")
                            nc.scalar.activation(
                                out=rms, in_=ssq,
                                func=mybir.ActivationFunctionType.Sqrt,
                                bias=eps_t, scale=1.0 / D,
                            )
                            rinv = normtmp.tile([128, TQ], F32, tag="rinv")
                            nc.vector.reciprocal_approx_fast(out=rinv, in_=rms)
                            rsc = normtmp.tile([128, TQ], F32, tag="rsc")
                            nc.vector.tensor_scalar_mul(
                                out=rsc, in0=rinv,
                                scalar1=qks_t[:, 0:1] if m < 4
                                else qks_t[:, 1:2],
                            )
                            dst = qTn[n][:, m, :] if m < 4 else kTn[n][:, :]
                            nc.vector.tensor_mul(dst, araw, rsc)
                        else:
                            # v block: transpose to natural [tk, d]
                            vt = vtmp.tile([128, TQ], MM_DT, tag="vt")
                            nc.vector.tensor_copy(vt, acc)
                            for jj in range(4):
                                vps = psV.tile([128, 128], MM_DT, tag="vps")
                                nc.tensor.transpose(
                                    vps, vt[:, jj * 128:(jj + 1) * 128], ident
                                )
                                nc.vector.tensor_copy(vn[n][:, jj, :], vps)

                    for n in range(NT):
                        xt_sb = xts[n]
                        if n + 1 < NT:
                            for g4 in range(4):
                                nc.sync.dma_start(
                                    out=xts[n + 1][:, 4 * g4:4 * g4 + 4, :],
                                    in_=xT_r[
                                        :, 4 * g4:4 * g4 + 4,
                                        (n + 1) * TQ:(n + 2) * TQ
                                    ],
                                )
                        if n == 1:
                            # wp prefetch: late enough not to delay xt tiles,
                            # early enough to be resident before proj starts
                            for h in range(NH):
                                nc.sync.dma_start(
                                    out=wp_sb[:, h, :], in_=wp_r[:, h, :]
                                )
                        if n == 0:
                            # tile 0 is DMA-paced: stream chunks into 6
                            # parallel accumulators so consumption matches
                            # per-chunk arrival order
                            accs = [
                                psA.tile(
                                    [128, TQ], F32,
                                    tag=f"acc{m}", name=f"acc{m}",
                                )
                                for m in range(6)
                            ]
                            for kc in range(NKC):
                                for m in range(6):
                                    nc.tensor.matmul(
                                        accs[m],
                                        lhsT=wg_sb[
                                            :, kc, m * 128:(m + 1) * 128
                                        ],
                                        rhs=xt_sb[:, kc, :],
                                        start=(kc == 0),
                                        stop=(kc == NKC - 1),
                                    )
                                if kc <= 3:
                                    # tile 0 is DMA-paced with ~zero slack
                                    # and the first chunks arrive slowest;
                                    # throwaway matmuls into the idle ssq
                                    # bank give the DMA stream a lead so
                                    # the PE never starves (a starve-gap
                                    # re-throttles the clock-gate)
                                    warm = psN.tile(
                                        [128, TQ], F32, tag="ssq"
                                    )
                                    for _ in range(2 if kc <= 1 else 1):
                                        nc.tensor.matmul(
                                            warm, lhsT=ident,
                                            rhs=xt_sb[:, kc, :],
                                            start=True, stop=True,
                                        )
                            for m in range(6):
                                process_m(n, m, accs[m])
                        else:
                            # tiles 1-3: m's in groups, so each group's
                            # norm chains drain (and free their PSUM banks)
                            # while the next group accumulates.  This also
                            # frees the attention-phase banks (reused from
                            # psA in order) progressively, killing the
                            # stage A -> attention stall that re-throttled
                            # the PE clock-gate.  The last tile splits
                            # (4,5) into two solo groups: m4's ACT chain
                            # and the Exp/Copy table warms drain during
                            # m5's accumulation, so ACT is free for the
                            # first exps when attention starts.
                            groups = (
                                [(0, 1), (2, 3), (4,), (5,)]
                                if n == NT - 1
                                else [(0, 1), (2, 3), (4, 5)]
                            )
                            for ms in groups:
                                accs = {
                                    m: psA.tile(
                                        [128, TQ], F32,
                                        tag=f"acc{m}", name=f"acc{m}",
                                    )
                                    for m in ms
                                }
                                for kc in range(NKC):
                                    for m in ms:
                                        nc.tensor.matmul(
                                            accs[m],
                                            lhsT=wg_sb[
                                                :, kc, m * 128:(m + 1) * 128
                                            ],
                                            rhs=xt_sb[:, kc, :],
                                            start=(kc == 0),
                                            stop=(kc == NKC - 1),
                                        )
                                for m in ms:
                                    process_m(n, m, accs[m])
                                if n == NT - 1 and ms == (4,):
                                    # warm the Exp and Copy table sets now
                                    # (evicting Square/Sqrt, which are done
                                    # for good); overlaps m5 accumulation
                                    for fn in (
                                        mybir.ActivationFunctionType.Exp,
                                        mybir.ActivationFunctionType.Copy,
                                    ):
                                        nc.scalar.activation(
                                            out=tblw, in_=eps_t, func=fn
                                        )

            # ============ Attention + proj, per 512-wide q tile ============
            with (
                tc.tile_pool(name="pt_pool", bufs=4) as pt_pool,
                tc.tile_pool(name="pa_pool", bufs=3) as pa_pool,
                tc.tile_pool(name="rs_pool", bufs=3) as rs_pool,
                tc.tile_pool(name="yp_pool", bufs=2) as yp_pool,
                tc.tile_pool(name="psS", bufs=2, space="PSUM") as psS,
                tc.tile_pool(name="psO", bufs=2, space="PSUM") as psO,
                tc.tile_pool(name="psC", bufs=2, space="PSUM") as psC,
            ):
                def emit_proj(q0, qw):
                    # y[window] = (oT).T @ wp for this query window
                    for tt in range(qw // 128):
                        c0 = q0 + tt * 128
                        n, no = c0 // TQ, c0 % TQ
                        # the final row tile streams out per-512-col piece
                        # so the last output DMA is a quarter of the size
                        split_dma = (c0 == T - 128)
                        yp = yp_pool.tile([128, C], BF16, tag="yp", name="yp")
                        for cn in range(4):
                            pc = psC.tile([128, TQ], F32, tag="pc", name="pc")
                            for h in range(NH):
                                nc.tensor.matmul(
                                    pc,
                                    lhsT=oTn[n][:, h, no:no + 128],
                                    rhs=wp_sb[:, h, cn * TQ:(cn + 1) * TQ],
                                    start=(h == 0),
                                    stop=(h == NH - 1),
                                )
                            # alternate copy engines so neither ACT nor DVE
                            # becomes the drain bottleneck; window (1536,)
                            # drains on ACT only (the DVE still has its
                            # p_acc/normalize chains), but the final window
                            # alternates again — at the very end both
                            # engines drain in parallel
                            if q0 == 1536 or cn % 2 == 0:
                                nc.scalar.copy(yp[:, cn * TQ:(cn + 1) * TQ], pc)
                            else:
                                nc.vector.tensor_copy(
                                    yp[:, cn * TQ:(cn + 1) * TQ], pc
                                )
                            if split_dma:
                                nc.sync.dma_start(
                                    out=out_d[
                                        c0:c0 + 128, cn * TQ:(cn + 1) * TQ
                                    ],
                                    in_=yp[:, cn * TQ:(cn + 1) * TQ],
                                )
                        if not split_dma:
                            nc.sync.dma_start(out=out_d[c0:c0 + 128, :], in_=yp)

                fin_q = []

                def finalize(item):
                    # one ones-matmul over the accumulated p_acc gives
                    # the softmax denominator (replaces the per-chunk row-sum
                    # matmuls), then normalize; emit proj for the window
                    # after its last head.
                    o_ps, p_acc, q0, qw, h = item
                    n, qo = q0 // TQ, q0 % TQ
                    # u shares the psC ring (saves a PSUM bank for psO=2;
                    # psS's ring parity must stay untouched)
                    u_ps = psC.tile([128, TQ], F32, tag="pc", name="u_ps")
                    nc.tensor.matmul(
                        u_ps[:, 0:qw], lhsT=ones_m, rhs=p_acc[:, 0:qw]
                    )
                    rsum = rs_pool.tile(
                        [128, TQ], F32, tag="rsum", name="rsum"
                    )
                    nc.vector.reciprocal_approx_fast(
                        out=rsum[:, 0:qw], in_=u_ps[:, 0:qw]
                    )
                    nc.vector.tensor_mul(
                        oTn[n][:, h, qo:qo + qw], o_ps[:, 0:qw], rsum[:, 0:qw]
                    )
                    if h == NH - 1:
                        emit_proj(q0, qw)

                def flush(item):
                    # PV matmuls for a finished exp batch. The head's
                    # finalize (denominator + normalize + proj) is deferred
                    # a full head so the exp->mask->p_acc-add chain of the
                    # diagonal batch is long done when the u matmul issues.
                    batch, p_sb, o_ps, p_acc, q0, qw, h, is_last, jmax = item
                    for (j, co, w, pos) in batch:
                        nc.tensor.matmul(
                            o_ps[:, co:co + w],
                            lhsT=vn[j // 4][:, j % 4, :],
                            rhs=p_sb[:, pos:pos + w],
                            start=(j == 0),
                            stop=(j == jmax),
                        )
                    # p_acc accumulation emitted here (2 batches after the
                    # exp/masks) so the DVE queue never sits waiting on a
                    # just-issued exp or gpsimd mask — an inline wait stalls
                    # every DVE op queued behind it.  (Keep this on DVE:
                    # gpsimd tensor ops measured 2-6x slower and its FIFO
                    # would delay the causal masks that gate PV.)
                    for (j, co, w, pos) in batch:
                        if j == 0:
                            nc.vector.tensor_copy(
                                p_acc[:, 0:qw], p_sb[:, pos:pos + w]
                            )
                        else:
                            nc.vector.tensor_add(
                                p_acc[:, co:co + w],
                                p_acc[:, co:co + w],
                                p_sb[:, pos:pos + w],
                            )
                    if is_last:
                        if fin_q:
                            finalize(fin_q.pop(0))
                        fin_q.append((o_ps, p_acc, q0, qw, h))

                # depth-2 software pipeline: flush batch i-2 while batch i's
                # scores stream and batch i-1's exp runs on ACT, so the PV
                # matmuls never wait on a just-issued exp
                import collections
                pending = collections.deque()
                first_flush = [True]

                def fill_bridge(n_mm):
                    # throwaway matmuls into ONE psC slot (the first slot
                    # reuses psN's old bank, free well before the boundary;
                    # the second slot was psV's, which the v-transposes
                    # hold until the very end of stage A): keep the PE (and
                    # its HAM clock-gate) busy while the S->exp->PV
                    # pipeline fills or the final p_acc chains drain
                    dum = psC.tile([128, TQ], F32, tag="pc", name="pc")
                    for i in range(n_mm):
                        nc.tensor.matmul(
                            dum, lhsT=ones_m, rhs=kTn[0][:, :],
                            start=True, stop=True,
                        )

                for (q0, qw) in WINDOWS:
                    batches = _pack_batches(q0, qw)
                    jmax = (q0 + qw) // 128 - 1
                    n, qo = q0 // TQ, q0 % TQ
                    for h in range(NH):
                        o_ps = psO.tile([128, TQ], F32, tag="o", name="o_ps")
                        p_acc = pa_pool.tile(
                            [128, TQ], MM_DT, tag="pa", name="p_acc"
                        )
                        for bi, batch in enumerate(batches):
                            bw = batch[-1][3] + batch[-1][2]
                            s_ps = psS.tile(
                                [128, 1024], F32, tag="s", name="s_ps"
                            )
                            for (j, co, w, pos) in batch:
                                nc.tensor.matmul(
                                    s_ps[:, pos:pos + w],
                                    lhsT=kTn[j // 4][
                                        :, (j % 4) * 128:(j % 4 + 1) * 128
                                    ],
                                    rhs=qTn[n][:, h, qo + co:qo + qw],
                                )
                            p_sb = pt_pool.tile(
                                [128, 1024], MM_DT, tag="p", name="p_sb"
                            )
                            nc.scalar.activation(
                                out=p_sb[:, 0:bw], in_=s_ps[:, 0:bw],
                                func=mybir.ActivationFunctionType.Exp,
                                scale=SM_SCALE,
                            )
                            for (j, co, w, pos) in batch:
                                if 128 * j >= q0:
                                    # causal mask: zero p where col < row
                                    # (gpsimd: otherwise-idle engine)
                                    nc.gpsimd.affine_select(
                                        out=p_sb[:, pos:pos + 128],
                                        in_=p_sb[:, pos:pos + 128],
                                        pattern=[[1, 128]],
                                        channel_multiplier=-1, base=0,
                                        compare_op=mybir.AluOpType.is_ge,
                                        fill=0.0,
                                    )
                            pending.append((
                                batch, p_sb, o_ps, p_acc, q0, qw, h,
                                bi == len(batches) - 1, jmax,
                            ))
                            if len(pending) > 2:
                                if first_flush[0]:
                                    # bridge the attention pipeline fill:
                                    # the first PV waits S0+exp0 (~1.6us)
                                    first_flush[0] = False
                                    fill_bridge(4)
                                flush(pending.popleft())
                while pending:
                    flush(pending.popleft())
                # bridge the drain: the last u matmuls wait the final
                # p_acc chains on the DVE (~2.5us)
                fill_bridge(12)
                while fin_q:
                    finalize(fin_q.pop(0))

    nc.finalize()
    return nc


@functools.lru_cache(maxsize=1)
def _get_nc():
    return build_kernel()


def make_in_maps(x, W_qkv, W_proj, q_scale, k_scale):
    x = np.asarray(x, dtype=np.float32)
    W_qkv = np.asarray(W_qkv, dtype=np.float32)
    W_proj = np.asarray(W_proj, dtype=np.float32)
    q_scale = np.asarray(q_scale, dtype=np.float32)
    k_scale = np.asarray(k_scale, dtype=np.float32)

    import ml_dtypes

    bf16 = ml_dtypes.bfloat16
    qks = np.ascontiguousarray(
        np.stack([q_scale, k_scale], axis=1).astype(np.float32)
    )
    xT_by_batch = [np.ascontiguousarray(x[b].T).astype(bf16) for b in range(2)]
    in_maps = []
    for core in range(8):
        b, g = divmod(core, 4)
        wg_cols = np.concatenate(
            [
                W_qkv[:, 512 * g:512 * (g + 1)],
                W_qkv[:, 2048 + 128 * g:2048 + 128 * (g + 1)],
                W_qkv[:, 2560 + 128 * g:2560 + 128 * (g + 1)],
            ],
            axis=1,
        )
        # pack to [p, kc, q] so per-partition DRAM lines span kc chunks
        wg = np.ascontiguousarray(
            wg_cols.reshape(16, 128, 768).transpose(1, 0, 2).reshape(128, -1)
        ).astype(bf16)
        wp = np.ascontiguousarray(W_proj[512 * g:512 * (g + 1), :]).astype(bf16)
        in_maps.append(
            {"xT": xT_by_batch[b], "wg": wg, "wp": wp, "qks": qks}
        )
    return in_maps


def kernel(x, W_qkv, W_proj, q_scale, k_scale):
    nc = _get_nc()
    in_maps = make_in_maps(x, W_qkv, W_proj, q_scale, k_scale)
    res = run_bass_kernel_spmd(nc, in_maps, core_ids=list(range(8)))
    outs = [np.asarray(r["out"], dtype=np.float32) for r in res.results]
    y0 = outs[0] + outs[1] + outs[2] + outs[3]
    y1 = outs[4] + outs[5] + outs[6] + outs[7]
    return np.stack([y0, y1], axis=0).astype(np.float32)
